# revision 27
# baseline (speedup 1.0000x reference)
"""Multi-head attention (dense transformer block) on 8 TRN2 NeuronCores.

Problem: inp [8, 1024, 1024], w_qkv [1024, 3072], w_proj [1024, 1024],
biases (zeros). out = proj(softmax(QK^T/sqrt(hd)) V), H=16 heads, hd=64.

Sharding: pure data-parallel over batch — each of the 8 cores handles one
batch element with fully replicated weights (B == n_cores == 8, the
zero-communication specialization of "DP over batch + TP over heads").

Per-core pipeline (layouts chosen so every matmul contracts over the SBUF
partition dim and the softmax denominator falls out of the AV matmul):

  x^T  = transpose(x)                           (PE transpose)
  V    = x . w_v  as lhsT=x^T, rhs=w_v (f32r)  -> [tok, feat], stored bf16
         interleaved per head as [64 V cols | 1 ones col]
  per feature-tile ft (= head pair 2ft, 2ft+1), interleaved so ACT exp
  always overlaps independent PE work:
     Q^T[ft] = lhsT=w_q, rhs=x^T (f32r) -> bf16 [feat,tok]
     K^T[ft] likewise
     per head h in pair:
        S^T[k,q] = lhsT=K^T_h, rhs=Q^T_h (bf16, K=64)
        A^T = exp(SCALE * S^T)   (ACT, PSUM->SBUF, bf16)
        [O^T_h ; r_h] = lhsT=[V_h | ones], rhs=A^T (bf16)
        O^T_h *= 1/r_h : row->SBUF, reciprocal_approx_fast, GPSIMD
        partition_broadcast, fused (PSUM * bcast) -> bf16 O^T
  out = lhsT=O^T, rhs=w_proj (bf16) -> DMA out

b_qkv / b_proj are zeros by construction (spec fill=zeros); b_proj is
added on host anyway (exact no-op for zeros), b_qkv must be zero.
"""

import sys

import numpy as np

if "/opt/trn_rl_repo" not in sys.path:
    sys.path.insert(0, "/opt/trn_rl_repo")

import concourse.bass as bass
import concourse.mybir as mybir
import concourse.tile as tile
from concourse import bacc
from concourse.bass_utils import run_bass_kernel_spmd
from concourse.masks import make_identity

B = 8
N = 1024  # tokens
D = 1024  # model dim
H = 16  # heads
HD = 64  # head dim
SCALE = HD ** -0.5

F32 = mybir.dt.float32
F32R = mybir.dt.float32r
BF16 = mybir.dt.bfloat16

NT = N // 128  # 8 token tiles
DT = D // 128  # 8 feature tiles
VSTRIDE = HD + 1  # V columns per head incl. ones column
MULT = mybir.AluOpType.mult


def build_attention_core() -> bass.Bass:
    """One NeuronCore's program: full attention for one batch element."""
    nc = bacc.Bacc("TRN2", target_bir_lowering=False, debug=False)

    x_d = nc.declare_dram_parameter("inp", [N, D], F32, isOutput=False)
    wqkv_d = nc.declare_dram_parameter("w_qkv", [D, 3 * D], F32, isOutput=False)
    wp_d = nc.declare_dram_parameter("w_proj", [D, D], F32, isOutput=False)
    out_d = nc.declare_dram_parameter("out", [N, D], F32, isOutput=True)

    with tile.TileContext(nc) as tc:
        with tc.tile_pool(name="res", bufs=1) as res, tc.tile_pool(
            name="str", bufs=1
        ) as st, tc.tile_pool(name="ps", bufs=1, space="PSUM") as ps:
            # Resident tensors.
            QT = [res.tile([128, N], BF16, name=f"qt{i}") for i in range(DT)]
            KT = [res.tile([128, N], BF16, name=f"kt{i}") for i in range(DT)]
            OT = [res.tile([128, N], BF16, name=f"ot{i}") for i in range(DT)]
            Vaug = [
                res.tile([128, H * VSTRIDE], BF16, name=f"va{i}") for i in range(NT)
            ]
            wpb = [res.tile([128, N], BF16, name=f"wpb{i}") for i in range(DT)]
            ident = res.tile([128, 128], F32, name="ident")
            warm = res.tile([1, 16], F32, name="warm")

            make_identity(nc, ident)
            # Ones columns of Vaug; V data copies overwrite the rest later.
            for t in Vaug:
                nc.vector.memset(t, 1.0)
            # Trigger the exp table load early so it overlaps the DMAs.
            nc.vector.memset(warm, 0.0)
            nc.scalar.activation(warm, warm, mybir.ActivationFunctionType.Exp)

            xT = [st.tile([128, N], F32R, name=f"xt{i}") for i in range(DT)]

            # ---- x -> x^T via PE transpose ----
            for nt in range(NT):
                xin = st.tile([128, D], F32, name="xin", tag="xin", bufs=2)
                nc.sync.dma_start(out=xin, in_=x_d[nt * 128 : (nt + 1) * 128, :])
                for dt in range(DT):
                    ptr = ps.tile([128, 128], F32, name="ptr", tag="st", bufs=3)
                    nc.tensor.transpose(ptr, xin[:, dt * 128 : (dt + 1) * 128], ident)
                    nc.vector.tensor_copy(xT[dt][:, nt * 128 : (nt + 1) * 128], ptr)

            # ---- V (natural layout, bf16, interleaved with ones cols) ----
            for ch in range(2):
                wvs = []
                for kt in range(DT):
                    wv = st.tile([128, 512], F32R, name="wv", tag="wv", bufs=8)
                    nc.sync.dma_start(
                        out=wv,
                        in_=wqkv_d[
                            kt * 128 : (kt + 1) * 128,
                            2 * D + ch * 512 : 2 * D + (ch + 1) * 512,
                        ].bitcast(F32R),
                    )
                    wvs.append(wv)
                for nt in range(NT):
                    pv = ps.tile([128, 512], F32, name="pv", tag="av", bufs=2)
                    for kt in range(DT):
                        nc.tensor.matmul(
                            pv,
                            xT[kt][:, nt * 128 : (nt + 1) * 128],
                            wvs[kt],
                            start=(kt == 0),
                            stop=(kt == DT - 1),
                        )
                    dst3 = Vaug[nt].rearrange("p (h c) -> p h c", c=VSTRIDE)[
                        :, ch * 8 : (ch + 1) * 8, 0:HD
                    ]
                    src3 = pv.rearrange("p (h c) -> p h c", c=HD)
                    nc.vector.tensor_copy(dst3, src3)

            # ---- w_proj load + cast (overlaps attention via DMA/DVE) ----
            for dt in range(DT):
                wpf = st.tile([128, N], F32, name="wpf", tag="wpf", bufs=1)
                nc.sync.dma_start(out=wpf, in_=wp_d[dt * 128 : (dt + 1) * 128, :])
                nc.vector.tensor_copy(wpb[dt], wpf)

            # ---- merged QKV + attention, one feature-tile (head pair) at
            # a time so ACT exp overlaps independent PE matmuls ----
            for ft in range(DT):
                for which, base, dst in (("q", 0, QT), ("k", D, KT)):
                    wts = []
                    for kt in range(DT):
                        w = st.tile(
                            [128, 128], F32R, name=f"w{which}", tag="wqk", bufs=18
                        )
                        nc.sync.dma_start(
                            out=w,
                            in_=wqkv_d[
                                kt * 128 : (kt + 1) * 128,
                                base + ft * 128 : base + (ft + 1) * 128,
                            ].bitcast(F32R),
                        )
                        wts.append(w)
                    for ch in range(2):
                        sl = slice(ch * 512, (ch + 1) * 512)
                        pq = ps.tile([128, 512], F32, name="pq", tag="av", bufs=2)
                        for kt in range(DT):
                            nc.tensor.matmul(
                                pq,
                                wts[kt],
                                xT[kt][:, sl],
                                start=(kt == 0),
                                stop=(kt == DT - 1),
                            )
                        nc.vector.tensor_copy(dst[ft][:, sl], pq)

                for h in (2 * ft, 2 * ft + 1):
                    hr = (h % 2) * HD
                    ats = []
                    for kt in range(NT):
                        pss = ps.tile([128, N], F32, name="pss", tag="st", bufs=3)
                        for ch in range(2):
                            sl = slice(ch * 512, (ch + 1) * 512)
                            nc.tensor.matmul(
                                pss[:, sl],
                                KT[ft][hr : hr + HD, kt * 128 : (kt + 1) * 128],
                                QT[ft][hr : hr + HD, sl],
                                start=True,
                                stop=True,
                            )
                        at = st.tile([128, N], BF16, name="at", tag="at", bufs=12)
                        nc.scalar.activation(
                            at, pss, mybir.ActivationFunctionType.Exp, scale=SCALE
                        )
                        ats.append(at)
                    for ch in range(2):
                        sl = slice(ch * 512, (ch + 1) * 512)
                        po = ps.tile([HD + 1, 512], F32, name="po", tag="av", bufs=2)
                        for kt in range(NT):
                            nc.tensor.matmul(
                                po,
                                Vaug[kt][:, h * VSTRIDE : (h + 1) * VSTRIDE],
                                ats[kt][:, sl],
                                start=(kt == 0),
                                stop=(kt == NT - 1),
                            )
                        s64 = st.tile([1, 512], F32, name="s64", tag="s64", bufs=3)
                        nc.vector.tensor_copy(s64, po[HD : HD + 1, :])
                        rinv = st.tile([1, 512], F32, name="rinv", tag="rinv", bufs=3)
                        nc.vector.reciprocal_approx_fast(rinv, s64)
                        rb = st.tile([HD, 512], F32, name="rb", tag="rb", bufs=3)
                        nc.gpsimd.partition_broadcast(out_ap=rb, in_ap=rinv)
                        # OT slice = (po * 1.0) * rb  — one fused DVE op.
                        nc.vector.scalar_tensor_tensor(
                            out=OT[ft][hr : hr + HD, sl],
                            in0=po[0:HD, :],
                            scalar=1.0,
                            in1=rb,
                            op0=MULT,
                            op1=MULT,
                        )

            # ---- output projection ----
            for nt in range(NT):
                for ch in range(2):
                    sl = slice(ch * 512, (ch + 1) * 512)
                    pp = ps.tile([128, 512], F32, name="pp", tag="av", bufs=2)
                    for dt in range(DT):
                        nc.tensor.matmul(
                            pp,
                            OT[dt][:, nt * 128 : (nt + 1) * 128],
                            wpb[dt][:, sl],
                            start=(dt == 0),
                            stop=(dt == DT - 1),
                        )
                    ob = st.tile([128, 512], F32, name="ob", tag="ob", bufs=3)
                    nc.vector.tensor_copy(ob, pp)
                    nc.sync.dma_start(
                        out=out_d[nt * 128 : (nt + 1) * 128, sl], in_=ob
                    )

    nc.compile()
    return nc


_NC_CACHE = None


def _get_nc() -> bass.Bass:
    global _NC_CACHE
    if _NC_CACHE is None:
        _NC_CACHE = build_attention_core()
    return _NC_CACHE


def kernel(inp, w_qkv, b_qkv, w_proj, b_proj, _trace=False):
    inp = np.ascontiguousarray(np.asarray(inp, dtype=np.float32))
    w_qkv = np.ascontiguousarray(np.asarray(w_qkv, dtype=np.float32))
    w_proj = np.ascontiguousarray(np.asarray(w_proj, dtype=np.float32))
    b_qkv = np.asarray(b_qkv, dtype=np.float32)
    b_proj = np.asarray(b_proj, dtype=np.float32)
    assert inp.shape == (B, N, D)
    # The device kernel folds no qkv bias; the spec guarantees zeros.
    assert not np.any(b_qkv), "kernel assumes b_qkv == 0 (spec fill=zeros)"

    nc = _get_nc()
    in_maps = [
        {"inp": inp[b], "w_qkv": w_qkv, "w_proj": w_proj} for b in range(B)
    ]
    res = run_bass_kernel_spmd(nc, in_maps, core_ids=list(range(B)), trace=_trace)
    out = np.stack([res.results[b]["out"] for b in range(B)], axis=0)
    out = out + b_proj  # exact no-op for the spec's zero bias
    if _trace:
        return out.astype(np.float32), res
    return out.astype(np.float32)


# revision 29
# speedup vs baseline: 1.1812x; 1.1812x over previous
"""Multi-head attention (dense transformer block) on 8 TRN2 NeuronCores.

Problem: inp [8, 1024, 1024], w_qkv [1024, 3072], w_proj [1024, 1024],
biases (zeros). out = proj(softmax(QK^T/sqrt(hd)) V), H=16 heads, hd=64.

Sharding: pure data-parallel over batch — each of the 8 cores handles one
batch element with fully replicated weights (B == n_cores == 8, the
zero-communication specialization of "DP over batch + TP over heads").

Per-core pipeline (layouts chosen so every matmul contracts over the SBUF
partition dim and the softmax denominator falls out of the AV matmul):

  x^T  = transpose(x)                           (PE transpose)
  V    = x . w_v  as lhsT=x^T, rhs=w_v (f32r)  -> [tok, feat], stored bf16
         interleaved per head as [64 V cols | 1 ones col]
  per feature-tile ft (= head pair 2ft, 2ft+1), interleaved so ACT exp
  always overlaps independent PE work:
     Q^T[ft] = lhsT=w_q, rhs=x^T (f32r) -> bf16 [feat,tok]
     K^T[ft] likewise
     per head h in pair:
        S^T[k,q] = lhsT=K^T_h, rhs=Q^T_h (bf16, K=64)
        A^T = exp(SCALE * S^T)   (ACT, PSUM->SBUF, bf16)
        [O^T_h ; r_h] = lhsT=[V_h | ones], rhs=A^T (bf16)
        O^T_h *= 1/r_h : row->SBUF, reciprocal_approx_fast, GPSIMD
        partition_broadcast, fused (PSUM * bcast) -> bf16 O^T
  out = lhsT=O^T, rhs=w_proj (bf16) -> DMA out

b_qkv / b_proj are zeros by construction (spec fill=zeros); b_proj is
added on host anyway (exact no-op for zeros), b_qkv must be zero.
"""

import sys

import numpy as np

if "/opt/trn_rl_repo" not in sys.path:
    sys.path.insert(0, "/opt/trn_rl_repo")

import concourse.bass as bass
import concourse.mybir as mybir
import concourse.tile as tile
from concourse import bacc
from concourse.bass_utils import run_bass_kernel_spmd
from concourse.masks import make_identity

B = 8
N = 1024  # tokens
D = 1024  # model dim
H = 16  # heads
HD = 64  # head dim
SCALE = HD ** -0.5

F32 = mybir.dt.float32
F32R = mybir.dt.float32r
BF16 = mybir.dt.bfloat16

NT = N // 128  # 8 token tiles
DT = D // 128  # 8 feature tiles
VSTRIDE = HD + 1  # V columns per head incl. ones column
MULT = mybir.AluOpType.mult


def build_attention_core() -> bass.Bass:
    """One NeuronCore's program: full attention for one batch element."""
    nc = bacc.Bacc("TRN2", target_bir_lowering=False, debug=False)

    x_d = nc.declare_dram_parameter("inp", [N, D], F32, isOutput=False)
    wqkv_d = nc.declare_dram_parameter("w_qkv", [D, 3 * D], F32, isOutput=False)
    wp_d = nc.declare_dram_parameter("w_proj", [D, D], F32, isOutput=False)
    out_d = nc.declare_dram_parameter("out", [N, D], F32, isOutput=True)

    with tile.TileContext(nc) as tc:
        with tc.tile_pool(name="res", bufs=1) as res, tc.tile_pool(
            name="str", bufs=1
        ) as st, tc.tile_pool(name="ps", bufs=1, space="PSUM") as ps:
            # Resident tensors.
            QT = [res.tile([128, N], BF16, name=f"qt{i}") for i in range(DT)]
            KT = [res.tile([128, N], BF16, name=f"kt{i}") for i in range(DT)]
            OT = [res.tile([128, N], BF16, name=f"ot{i}") for i in range(DT)]
            Vaug = [
                res.tile([128, H * VSTRIDE], BF16, name=f"va{i}") for i in range(NT)
            ]
            wpb = [res.tile([128, N], BF16, name=f"wpb{i}") for i in range(DT)]
            ident = res.tile([128, 128], F32, name="ident")
            warm = res.tile([1, 16], F32, name="warm")

            make_identity(nc, ident)
            # Ones columns of Vaug; V data copies overwrite the rest later.
            for t in Vaug:
                nc.vector.memset(t, 1.0)
            # Trigger the exp table load early so it overlaps the DMAs.
            nc.vector.memset(warm, 0.0)
            nc.scalar.activation(warm, warm, mybir.ActivationFunctionType.Exp)

            xT = [st.tile([128, N], F32R, name=f"xt{i}") for i in range(DT)]

            # V weights for the first chunk, prefetched alongside x.
            wvs0 = []
            for kt in range(DT):
                wv = st.tile([128, 512], F32R, name="wv", tag="wv", bufs=9)
                nc.sync.dma_start(
                    out=wv,
                    in_=wqkv_d[
                        kt * 128 : (kt + 1) * 128, 2 * D : 2 * D + 512
                    ].bitcast(F32R),
                )
                wvs0.append(wv)

            # ---- x -> x^T via PE transpose; V(nt, ch0) follows each tile's
            # transposes so V overlaps the transpose/DMA phase ----
            def v_chunk(nt, ch, wvs):
                pv = ps.tile([128, 512], F32, name="pv", tag="av", bufs=4)
                for kt in range(DT):
                    nc.tensor.matmul(
                        pv,
                        xT[kt][:, nt * 128 : (nt + 1) * 128],
                        wvs[kt],
                        start=(kt == 0),
                        stop=(kt == DT - 1),
                    )
                dst3 = Vaug[nt].rearrange("p (h c) -> p h c", c=VSTRIDE)[
                    :, ch * 8 : (ch + 1) * 8, 0:HD
                ]
                src3 = pv.rearrange("p (h c) -> p h c", c=HD)
                nc.vector.tensor_copy(dst3, src3)

            for nt in range(NT):
                xin = st.tile([128, D], F32, name="xin", tag="xin", bufs=2)
                nc.sync.dma_start(out=xin, in_=x_d[nt * 128 : (nt + 1) * 128, :])
                for dt in range(DT):
                    ptr = ps.tile([128, 128], F32, name="ptr", tag="st", bufs=2)
                    nc.tensor.transpose(ptr, xin[:, dt * 128 : (dt + 1) * 128], ident)
                    nc.vector.tensor_copy(xT[dt][:, nt * 128 : (nt + 1) * 128], ptr)
                v_chunk(nt, 0, wvs0)

            # ---- V second chunk ----
            wvs1 = []
            for kt in range(DT):
                wv = st.tile([128, 512], F32R, name="wv", tag="wv", bufs=9)
                nc.sync.dma_start(
                    out=wv,
                    in_=wqkv_d[
                        kt * 128 : (kt + 1) * 128, 2 * D + 512 : 3 * D
                    ].bitcast(F32R),
                )
                wvs1.append(wv)
            for nt in range(NT):
                v_chunk(nt, 1, wvs1)

            # ---- w_proj load + cast (overlaps attention via DMA/DVE) ----
            for dt in range(DT):
                wpf = st.tile([128, N], F32, name="wpf", tag="wpf", bufs=1)
                nc.sync.dma_start(out=wpf, in_=wp_d[dt * 128 : (dt + 1) * 128, :])
                nc.vector.tensor_copy(wpb[dt], wpf)

            # ---- merged QKV + attention, one feature-tile (head pair) at
            # a time so ACT exp overlaps independent PE matmuls ----
            for ft in range(DT):
                for which, base, dst in (("q", 0, QT), ("k", D, KT)):
                    wts = []
                    for kt in range(DT):
                        w = st.tile(
                            [128, 128], F32R, name=f"w{which}", tag="wqk", bufs=18
                        )
                        nc.sync.dma_start(
                            out=w,
                            in_=wqkv_d[
                                kt * 128 : (kt + 1) * 128,
                                base + ft * 128 : base + (ft + 1) * 128,
                            ].bitcast(F32R),
                        )
                        wts.append(w)
                    for ch in range(2):
                        sl = slice(ch * 512, (ch + 1) * 512)
                        pq = ps.tile([128, 512], F32, name="pq", tag="av", bufs=4)
                        for kt in range(DT):
                            nc.tensor.matmul(
                                pq,
                                wts[kt],
                                xT[kt][:, sl],
                                start=(kt == 0),
                                stop=(kt == DT - 1),
                            )
                        nc.vector.tensor_copy(dst[ft][:, sl], pq)

                for h in (2 * ft, 2 * ft + 1):
                    hr = (h % 2) * HD
                    ats = []
                    for kt in range(NT):
                        pss = ps.tile([128, N], F32, name="pss", tag="st", bufs=2)
                        for ch in range(2):
                            sl = slice(ch * 512, (ch + 1) * 512)
                            nc.tensor.matmul(
                                pss[:, sl],
                                KT[ft][hr : hr + HD, kt * 128 : (kt + 1) * 128],
                                QT[ft][hr : hr + HD, sl],
                                start=True,
                                stop=True,
                            )
                        at = st.tile([128, N], BF16, name="at", tag="at", bufs=12)
                        nc.scalar.activation(
                            at, pss, mybir.ActivationFunctionType.Exp, scale=SCALE
                        )
                        ats.append(at)
                    for ch in range(2):
                        sl = slice(ch * 512, (ch + 1) * 512)
                        po = ps.tile([HD + 1, 512], F32, name="po", tag="av", bufs=4)
                        for kt in range(NT):
                            nc.tensor.matmul(
                                po,
                                Vaug[kt][:, h * VSTRIDE : (h + 1) * VSTRIDE],
                                ats[kt][:, sl],
                                start=(kt == 0),
                                stop=(kt == NT - 1),
                            )
                        s64 = st.tile([1, 512], F32, name="s64", tag="s64", bufs=3)
                        nc.vector.tensor_copy(s64, po[HD : HD + 1, :])
                        rinv = st.tile([1, 512], F32, name="rinv", tag="rinv", bufs=3)
                        nc.vector.reciprocal_approx_fast(rinv, s64)
                        rb = st.tile([HD, 512], F32, name="rb", tag="rb", bufs=3)
                        nc.gpsimd.partition_broadcast(out_ap=rb, in_ap=rinv)
                        # OT slice = (po * 1.0) * rb  — one fused DVE op.
                        nc.vector.scalar_tensor_tensor(
                            out=OT[ft][hr : hr + HD, sl],
                            in0=po[0:HD, :],
                            scalar=1.0,
                            in1=rb,
                            op0=MULT,
                            op1=MULT,
                        )

            # ---- output projection ----
            for nt in range(NT):
                for ch in range(2):
                    sl = slice(ch * 512, (ch + 1) * 512)
                    pp = ps.tile([128, 512], F32, name="pp", tag="av", bufs=4)
                    for dt in range(DT):
                        nc.tensor.matmul(
                            pp,
                            OT[dt][:, nt * 128 : (nt + 1) * 128],
                            wpb[dt][:, sl],
                            start=(dt == 0),
                            stop=(dt == DT - 1),
                        )
                    ob = st.tile([128, 512], F32, name="ob", tag="ob", bufs=3)
                    nc.vector.tensor_copy(ob, pp)
                    nc.sync.dma_start(
                        out=out_d[nt * 128 : (nt + 1) * 128, sl], in_=ob
                    )

    nc.compile()
    return nc


_NC_CACHE = None


def _get_nc() -> bass.Bass:
    global _NC_CACHE
    if _NC_CACHE is None:
        _NC_CACHE = build_attention_core()
    return _NC_CACHE


def kernel(inp, w_qkv, b_qkv, w_proj, b_proj, _trace=False):
    inp = np.ascontiguousarray(np.asarray(inp, dtype=np.float32))
    w_qkv = np.ascontiguousarray(np.asarray(w_qkv, dtype=np.float32))
    w_proj = np.ascontiguousarray(np.asarray(w_proj, dtype=np.float32))
    b_qkv = np.asarray(b_qkv, dtype=np.float32)
    b_proj = np.asarray(b_proj, dtype=np.float32)
    assert inp.shape == (B, N, D)
    # The device kernel folds no qkv bias; the spec guarantees zeros.
    assert not np.any(b_qkv), "kernel assumes b_qkv == 0 (spec fill=zeros)"

    nc = _get_nc()
    in_maps = [
        {"inp": inp[b], "w_qkv": w_qkv, "w_proj": w_proj} for b in range(B)
    ]
    res = run_bass_kernel_spmd(nc, in_maps, core_ids=list(range(B)), trace=_trace)
    out = np.stack([res.results[b]["out"] for b in range(B)], axis=0)
    out = out + b_proj  # exact no-op for the spec's zero bias
    if _trace:
        return out.astype(np.float32), res
    return out.astype(np.float32)


# revision 30
# speedup vs baseline: 1.2437x; 1.0529x over previous
"""Multi-head attention (dense transformer block) on 8 TRN2 NeuronCores.

Problem: inp [8, 1024, 1024], w_qkv [1024, 3072], w_proj [1024, 1024],
biases (zeros). out = proj(softmax(QK^T/sqrt(hd)) V), H=16 heads, hd=64.

Sharding: pure data-parallel over batch — each of the 8 cores handles one
batch element with fully replicated weights (B == n_cores == 8, the
zero-communication specialization of "DP over batch + TP over heads").

Per-core pipeline (layouts chosen so every matmul contracts over the SBUF
partition dim and the softmax denominator falls out of the AV matmul):

  x^T  = transpose(x)                           (PE transpose)
  V    = x . w_v  as lhsT=x^T, rhs=w_v (f32r)  -> [tok, feat], stored bf16
         interleaved per head as [64 V cols | 1 ones col]
  per feature-tile ft (= head pair 2ft, 2ft+1), interleaved so ACT exp
  always overlaps independent PE work:
     Q^T[ft] = lhsT=w_q, rhs=x^T (f32r) -> bf16 [feat,tok]
     K^T[ft] likewise
     per head h in pair:
        S^T[k,q] = lhsT=K^T_h, rhs=Q^T_h (bf16, K=64)
        A^T = exp(SCALE * S^T)   (ACT, PSUM->SBUF, bf16)
        [O^T_h ; r_h] = lhsT=[V_h | ones], rhs=A^T (bf16)
        O^T_h *= 1/r_h : row->SBUF, reciprocal_approx_fast, GPSIMD
        partition_broadcast, fused (PSUM * bcast) -> bf16 O^T
  out = lhsT=O^T, rhs=w_proj (bf16) -> DMA out

b_qkv / b_proj are zeros by construction (spec fill=zeros); b_proj is
added on host anyway (exact no-op for zeros), b_qkv must be zero.
"""

import sys

import numpy as np

if "/opt/trn_rl_repo" not in sys.path:
    sys.path.insert(0, "/opt/trn_rl_repo")

import concourse.bass as bass
import concourse.mybir as mybir
import concourse.tile as tile
from concourse import bacc
from concourse.bass_utils import run_bass_kernel_spmd
from concourse.masks import make_identity

B = 8
N = 1024  # tokens
D = 1024  # model dim
H = 16  # heads
HD = 64  # head dim
SCALE = HD ** -0.5

F32 = mybir.dt.float32
F32R = mybir.dt.float32r
BF16 = mybir.dt.bfloat16

NT = N // 128  # 8 token tiles
DT = D // 128  # 8 feature tiles
VSTRIDE = HD + 1  # V columns per head incl. ones column
MULT = mybir.AluOpType.mult


def build_attention_core() -> bass.Bass:
    """One NeuronCore's program: full attention for one batch element."""
    nc = bacc.Bacc("TRN2", target_bir_lowering=False, debug=False)

    x_d = nc.declare_dram_parameter("inp", [N, D], F32, isOutput=False)
    wqkv_d = nc.declare_dram_parameter("w_qkv", [D, 3 * D], F32, isOutput=False)
    wp_d = nc.declare_dram_parameter("w_proj", [D, D], F32, isOutput=False)
    out_d = nc.declare_dram_parameter("out", [N, D], F32, isOutput=True)

    with tile.TileContext(nc) as tc:
        with tc.tile_pool(name="res", bufs=1) as res, tc.tile_pool(
            name="str", bufs=1
        ) as st, tc.tile_pool(name="ps", bufs=1, space="PSUM") as ps:
            # Resident tensors.
            QT = [res.tile([128, N], BF16, name=f"qt{i}") for i in range(DT)]
            KT = [res.tile([128, N], BF16, name=f"kt{i}") for i in range(DT)]
            OT = [res.tile([128, N], BF16, name=f"ot{i}") for i in range(DT)]
            Vaug = [
                res.tile([128, H * VSTRIDE], BF16, name=f"va{i}") for i in range(NT)
            ]
            wpb = [res.tile([128, N], BF16, name=f"wpb{i}") for i in range(DT)]
            ident = res.tile([128, 128], F32, name="ident")
            warm = res.tile([1, 16], F32, name="warm")

            make_identity(nc, ident)
            # Ones columns of Vaug; V data copies overwrite the rest later.
            for t in Vaug:
                nc.vector.memset(t, 1.0)
            # Trigger the exp table load early so it overlaps the DMAs.
            nc.vector.memset(warm, 0.0)
            nc.scalar.activation(warm, warm, mybir.ActivationFunctionType.Exp)

            xT = [st.tile([128, N], F32R, name=f"xt{i}") for i in range(DT)]

            # V weights for the first chunk, prefetched alongside x.
            wvs0 = []
            for kt in range(DT):
                wv = st.tile([128, 512], F32R, name="wv", tag="wv", bufs=9)
                nc.sync.dma_start(
                    out=wv,
                    in_=wqkv_d[
                        kt * 128 : (kt + 1) * 128, 2 * D : 2 * D + 512
                    ].bitcast(F32R),
                )
                wvs0.append(wv)

            # ---- x -> x^T via PE transpose; V(nt, ch0) follows each tile's
            # transposes so V overlaps the transpose/DMA phase ----
            def v_chunk(nt, ch, wvs):
                pv = ps.tile([128, 512], F32, name="pv", tag="av", bufs=4)
                for kt in range(DT):
                    nc.tensor.matmul(
                        pv,
                        xT[kt][:, nt * 128 : (nt + 1) * 128],
                        wvs[kt],
                        start=(kt == 0),
                        stop=(kt == DT - 1),
                    )
                dst3 = Vaug[nt].rearrange("p (h c) -> p h c", c=VSTRIDE)[
                    :, ch * 8 : (ch + 1) * 8, 0:HD
                ]
                src3 = pv.rearrange("p (h c) -> p h c", c=HD)
                nc.vector.tensor_copy(dst3, src3)

            for nt in range(NT):
                xin = st.tile([128, D], F32, name="xin", tag="xin", bufs=2)
                nc.sync.dma_start(out=xin, in_=x_d[nt * 128 : (nt + 1) * 128, :])
                for dt in range(DT):
                    ptr = ps.tile([128, 128], F32, name="ptr", tag="st", bufs=2)
                    nc.tensor.transpose(ptr, xin[:, dt * 128 : (dt + 1) * 128], ident)
                    nc.vector.tensor_copy(xT[dt][:, nt * 128 : (nt + 1) * 128], ptr)
                v_chunk(nt, 0, wvs0)

            # ---- V second chunk ----
            wvs1 = []
            for kt in range(DT):
                wv = st.tile([128, 512], F32R, name="wv", tag="wv", bufs=9)
                nc.sync.dma_start(
                    out=wv,
                    in_=wqkv_d[
                        kt * 128 : (kt + 1) * 128, 2 * D + 512 : 3 * D
                    ].bitcast(F32R),
                )
                wvs1.append(wv)
            for nt in range(NT):
                v_chunk(nt, 1, wvs1)

            # ---- w_proj load + cast (overlaps attention via DMA/DVE) ----
            for dt in range(DT):
                wpf = st.tile([128, N], F32, name="wpf", tag="wpf", bufs=1)
                nc.sync.dma_start(out=wpf, in_=wp_d[dt * 128 : (dt + 1) * 128, :])
                nc.vector.tensor_copy(wpb[dt], wpf)

            # ---- merged QKV + attention, one feature-tile (head pair) at
            # a time so ACT exp overlaps independent PE matmuls. QKV of
            # tile ft+1 is emitted between the two heads of tile ft so the
            # scheduler has adjacent filler for exp-paced stall regions. ----
            def qkv_tile(ft):
                for which, base, dst in (("q", 0, QT), ("k", D, KT)):
                    wts = []
                    for kt in range(DT):
                        w = st.tile(
                            [128, 128], F32R, name=f"w{which}", tag="wqk", bufs=18
                        )
                        nc.sync.dma_start(
                            out=w,
                            in_=wqkv_d[
                                kt * 128 : (kt + 1) * 128,
                                base + ft * 128 : base + (ft + 1) * 128,
                            ].bitcast(F32R),
                        )
                        wts.append(w)
                    for ch in range(2):
                        sl = slice(ch * 512, (ch + 1) * 512)
                        pq = ps.tile([128, 512], F32, name="pq", tag="av", bufs=4)
                        for kt in range(DT):
                            nc.tensor.matmul(
                                pq,
                                wts[kt],
                                xT[kt][:, sl],
                                start=(kt == 0),
                                stop=(kt == DT - 1),
                            )
                        nc.vector.tensor_copy(dst[ft][:, sl], pq)

            qkv_tile(0)
            for ft in range(DT):
                for h in (2 * ft, 2 * ft + 1):
                    if h % 2 == 1 and ft + 1 < DT:
                        qkv_tile(ft + 1)
                    hr = (h % 2) * HD
                    ats = []
                    for kt in range(NT):
                        pss = ps.tile([128, N], F32, name="pss", tag="st", bufs=2)
                        for ch in range(2):
                            sl = slice(ch * 512, (ch + 1) * 512)
                            nc.tensor.matmul(
                                pss[:, sl],
                                KT[ft][hr : hr + HD, kt * 128 : (kt + 1) * 128],
                                QT[ft][hr : hr + HD, sl],
                                start=True,
                                stop=True,
                            )
                        at = st.tile([128, N], BF16, name="at", tag="at", bufs=12)
                        nc.scalar.activation(
                            at, pss, mybir.ActivationFunctionType.Exp, scale=SCALE
                        )
                        ats.append(at)
                    for ch in range(2):
                        sl = slice(ch * 512, (ch + 1) * 512)
                        po = ps.tile([HD + 1, 512], F32, name="po", tag="av", bufs=4)
                        for kt in range(NT):
                            nc.tensor.matmul(
                                po,
                                Vaug[kt][:, h * VSTRIDE : (h + 1) * VSTRIDE],
                                ats[kt][:, sl],
                                start=(kt == 0),
                                stop=(kt == NT - 1),
                            )
                        s64 = st.tile([1, 512], F32, name="s64", tag="s64", bufs=3)
                        nc.vector.tensor_copy(s64, po[HD : HD + 1, :])
                        rinv = st.tile([1, 512], F32, name="rinv", tag="rinv", bufs=3)
                        nc.vector.reciprocal_approx_fast(rinv, s64)
                        rb = st.tile([HD, 512], F32, name="rb", tag="rb", bufs=3)
                        nc.gpsimd.partition_broadcast(out_ap=rb, in_ap=rinv)
                        # OT slice = (po * 1.0) * rb  — one fused DVE op.
                        nc.vector.scalar_tensor_tensor(
                            out=OT[ft][hr : hr + HD, sl],
                            in0=po[0:HD, :],
                            scalar=1.0,
                            in1=rb,
                            op0=MULT,
                            op1=MULT,
                        )

            # ---- output projection ----
            for nt in range(NT):
                for ch in range(2):
                    sl = slice(ch * 512, (ch + 1) * 512)
                    pp = ps.tile([128, 512], F32, name="pp", tag="av", bufs=4)
                    for dt in range(DT):
                        nc.tensor.matmul(
                            pp,
                            OT[dt][:, nt * 128 : (nt + 1) * 128],
                            wpb[dt][:, sl],
                            start=(dt == 0),
                            stop=(dt == DT - 1),
                        )
                    ob = st.tile([128, 512], F32, name="ob", tag="ob", bufs=3)
                    nc.vector.tensor_copy(ob, pp)
                    nc.sync.dma_start(
                        out=out_d[nt * 128 : (nt + 1) * 128, sl], in_=ob
                    )

    nc.compile()
    return nc


_NC_CACHE = None


def _get_nc() -> bass.Bass:
    global _NC_CACHE
    if _NC_CACHE is None:
        _NC_CACHE = build_attention_core()
    return _NC_CACHE


def kernel(inp, w_qkv, b_qkv, w_proj, b_proj, _trace=False):
    inp = np.ascontiguousarray(np.asarray(inp, dtype=np.float32))
    w_qkv = np.ascontiguousarray(np.asarray(w_qkv, dtype=np.float32))
    w_proj = np.ascontiguousarray(np.asarray(w_proj, dtype=np.float32))
    b_qkv = np.asarray(b_qkv, dtype=np.float32)
    b_proj = np.asarray(b_proj, dtype=np.float32)
    assert inp.shape == (B, N, D)
    # The device kernel folds no qkv bias; the spec guarantees zeros.
    assert not np.any(b_qkv), "kernel assumes b_qkv == 0 (spec fill=zeros)"

    nc = _get_nc()
    in_maps = [
        {"inp": inp[b], "w_qkv": w_qkv, "w_proj": w_proj} for b in range(B)
    ]
    res = run_bass_kernel_spmd(nc, in_maps, core_ids=list(range(B)), trace=_trace)
    out = np.stack([res.results[b]["out"] for b in range(B)], axis=0)
    out = out + b_proj  # exact no-op for the spec's zero bias
    if _trace:
        return out.astype(np.float32), res
    return out.astype(np.float32)
